# revision 6
# baseline (speedup 1.0000x reference)
"""Single-head causal attention (B=4, S=4096, E=768, H=64) on 8 TRN2 cores.

Sharding: core c handles batch b=c//2, sequence half h=c%2 (2048 query rows).
Each core receives x[b]^T with its own half first: positions 0..2047 are its
query rows, positions 2048..4095 are the other half.  The other half is a
fully-valid prefix for h=1 (past keys) and fully-masked for h=0 (future keys),
selected by a per-core bias vector fed to the exp.  This makes the program
identical on every core (single SPMD NEFF) while covering the causal split.

Compute layout (per core):
  phase A: K^T,V^T = [wk|wv]^T ë x^T (one packed pass), Q^T for own rows;
           V transposed to natural layout (+ ones column -> V_aug) via PE.
  phase B: per 512-query block, per 128-key chunk: S^T = K_chunk^T.T @ Q^T
           (PSUM), + causal mask on diagonal chunks, exp on ACT -> P^T in
           SBUF, then [V|1]^T.T-style accumulation out^T_aug = V_aug.T @ P^T
           (row 64 = softmax denominator).  Tail: PE-transpose, normalize.
All matmuls run as float32r (4x faster than fp32 on TRN2 PE).
"""

import numpy as np

import concourse.bass as bass
import concourse.tile as tile
from concourse import bacc, mybir, bass_utils
from concourse.masks import make_identity

F32 = mybir.dt.float32
F32R = mybir.dt.float32r
AF = mybir.ActivationFunctionType

B, S, E, H = 4, 4096, 768, 64
L = S // 2          # own rows per core
EC = E // 128       # e-chunks (6)
NSB = S // 512      # s-blocks over all positions (8)
NQB = L // 512      # q-blocks over own rows (4)
NKC = S // 128      # k-chunks over all positions (32)
NEG = -1.0e4


def build_nc(reps=None):
    nc = bacc.Bacc("TRN2", target_bir_lowering=False, debug=False, num_devices=8)
    xt = nc.dram_tensor("xt", [E, S], F32R, kind="ExternalInput").ap()
    wkv = nc.dram_tensor("wkv", [E, 2 * H], F32R, kind="ExternalInput").ap()
    wq = nc.dram_tensor("wq", [E, H], F32R, kind="ExternalInput").ap()
    bkv = nc.dram_tensor("bkv", [2 * H, 1], F32, kind="ExternalInput").ap()
    bq8 = nc.dram_tensor("bq8", [H, 1], F32, kind="ExternalInput").ap()
    pbias = nc.dram_tensor("pbias", [128, 1], F32, kind="ExternalInput").ap()
    r_out = nc.dram_tensor("r_out", [L, H], F32, kind="ExternalOutput").ap()
    k_out = nc.dram_tensor("k_out", [L, H], F32, kind="ExternalOutput").ap()
    v_out = nc.dram_tensor("v_out", [L, H], F32, kind="ExternalOutput").ap()

    xt_r = xt.rearrange("(c p) s -> p c s", p=128)
    wkv_r = wkv.rearrange("(c p) h -> p c h", p=128)
    wq_r = wq.rearrange("(c p) h -> p c h", p=128)

    with tile.TileContext(nc) as tc:
        with (
            tc.tile_pool(name="consts", bufs=1) as consts,
            tc.tile_pool(name="persist", bufs=1) as persist,
        ):
            # ---- constants ----
            wkv_sb = consts.tile([128, EC, 2 * H], F32R)
            nc.sync.dma_start(out=wkv_sb, in_=wkv_r)
            wq_sb = consts.tile([128, EC, H], F32R)
            nc.sync.dma_start(out=wq_sb, in_=wq_r)
            bkv_sb = consts.tile([2 * H, 1], F32)
            nc.sync.dma_start(out=bkv_sb, in_=bkv)
            bq8_sb = consts.tile([H, 1], F32)
            nc.sync.dma_start(out=bq8_sb, in_=bq8)
            pb_sb = consts.tile([128, 1], F32)
            nc.sync.dma_start(out=pb_sb, in_=pbias)
            ident = consts.tile([128, 128], F32)
            make_identity(nc, ident)
            masks = []
            for j in range(4):
                mk = consts.tile([128, 512], F32, tag=f"mask{j}")
                nc.gpsimd.memset(mk, 0.0)
                # valid (keep 0) iff f >= j*128 + p, else fill NEG
                nc.gpsimd.affine_select(
                    out=mk, in_=mk, compare_op=mybir.AluOpType.is_ge,
                    fill=NEG, base=-j * 128, pattern=[[1, 512]],
                    channel_multiplier=-1,
                )
                masks.append(mk)

            # ---- persistent per-iteration state ----
            kt = persist.tile([H, S], F32R)          # K^T over all positions
            qt = persist.tile([H, L], F32R)          # Q^T over own rows
            vaug = persist.tile([128, NKC, H + 1], F32R)  # V natural + ones col
            ones_f32 = consts.tile([128, NKC], F32)
            nc.vector.memset(ones_f32, 1.0)
            nc.vector.tensor_copy(vaug[:, :, H], ones_f32)

            def body():
                # ================= phase A: projections =================
                with (
                    tc.tile_pool(name="xt_pool", bufs=3) as xt_pool,
                    tc.tile_pool(name="vt_pool", bufs=2) as vt_pool,
                    tc.tile_pool(name="kf_pool", bufs=2) as kf_pool,
                    tc.tile_pool(name="st_pool", bufs=2) as st_pool,
                    tc.tile_pool(name="ps_proj", bufs=2, space="PSUM") as ps_proj,
                    tc.tile_pool(name="ps_q", bufs=2, space="PSUM") as ps_q,
                    tc.tile_pool(name="ps_t", bufs=3, space="PSUM") as ps_t,
                ):
                    for sb in range(NSB):
                        s0 = sb * 512
                        own = sb < NQB
                        xt_t = xt_pool.tile([128, EC, 512], F32R, tag="xt")
                        nc.sync.dma_start(out=xt_t, in_=xt_r[:, :, s0:s0 + 512])
                        psk = ps_proj.tile([128, 512], F32, tag="psk")
                        for c in range(EC):
                            nc.tensor.matmul(
                                psk, wkv_sb[:, c, :], xt_t[:, c, :],
                                start=(c == 0), stop=(c == EC - 1),
                            )
                        # K^T slice (f32r, biased)
                        nc.scalar.activation(
                            kt[:, s0:s0 + 512], psk[0:H, :], AF.Identity,
                            bias=bkv_sb[0:H, :], scale=1.0,
                        )
                        # V^T staging (f32, biased) -> transpose to natural
                        vt_t = vt_pool.tile([H, 512], F32, tag="vt")
                        nc.scalar.activation(
                            vt_t, psk[H:2 * H, :], AF.Identity,
                            bias=bkv_sb[H:2 * H, :], scale=1.0,
                        )
                        vstage = st_pool.tile([128, 4, H], F32, tag="vs")
                        for j in range(4):
                            pst = ps_t.tile([128, H], F32, tag="pst")
                            nc.tensor.transpose(
                                pst, vt_t[:, j * 128:(j + 1) * 128],
                                ident[0:H, 0:H],
                            )
                            nc.vector.tensor_copy(
                                vaug[:, sb * 4 + j, 0:H], pst)
                            if own:
                                nc.vector.tensor_copy(vstage[:, j, :], pst)
                        if own:
                            nc.sync.dma_start(
                                out=v_out[s0:s0 + 512, :].rearrange(
                                    "(j p) h -> p j h", p=128),
                                in_=vstage,
                            )
                            # Q^T (f32r, biased, pre-scaled by 1/8)
                            psq = ps_q.tile([H, 512], F32, tag="psq")
                            for c in range(EC):
                                nc.tensor.matmul(
                                    psq, wq_sb[:, c, :], xt_t[:, c, :],
                                    start=(c == 0), stop=(c == EC - 1),
                                )
                            nc.scalar.activation(
                                qt[:, s0:s0 + 512], psq, AF.Identity,
                                bias=bq8_sb, scale=0.125,
                            )
                            # K natural output (f32 path)
                            ktf = kf_pool.tile([H, 512], F32, tag="ktf")
                            nc.scalar.activation(
                                ktf, psk[0:H, :], AF.Identity,
                                bias=bkv_sb[0:H, :], scale=1.0,
                            )
                            kstage = st_pool.tile([128, 4, H], F32, tag="ks")
                            for j in range(4):
                                pst = ps_t.tile([128, H], F32, tag="pst")
                                nc.tensor.transpose(
                                    pst, ktf[:, j * 128:(j + 1) * 128],
                                    ident[0:H, 0:H],
                                )
                                nc.vector.tensor_copy(kstage[:, j, :], pst)
                            nc.sync.dma_start(
                                out=k_out[s0:s0 + 512, :].rearrange(
                                    "(j p) h -> p j h", p=128),
                                in_=kstage,
                            )

                # ================= phase B: attention =================
                with (
                    tc.tile_pool(name="pt_pool", bufs=4) as pt_pool,
                    tc.tile_pool(name="ob_pool", bufs=2) as ob_pool,
                    tc.tile_pool(name="rn_pool", bufs=2) as rn_pool,
                    tc.tile_pool(name="ro_pool", bufs=2) as ro_pool,
                    tc.tile_pool(name="rv_pool", bufs=2) as rv_pool,
                    tc.tile_pool(name="ps_s", bufs=4, space="PSUM") as ps_s,
                    tc.tile_pool(name="ps_o", bufs=2, space="PSUM") as ps_o,
                    tc.tile_pool(name="ps_t2", bufs=2, space="PSUM") as ps_t2,
                ):
                    for li in range(NQB):
                        qsl = qt[:, li * 512:(li + 1) * 512]
                        pso = ps_o.tile([H + 1, 512], F32, tag="pso")
                        chunks = list(range((li + 1) * 4)) + list(range(16, 32))
                        for idx, c in enumerate(chunks):
                            pss = ps_s.tile([128, 512], F32, tag="pss")
                            nc.tensor.matmul(
                                pss, kt[:, c * 128:(c + 1) * 128], qsl,
                                start=True, stop=True,
                            )
                            j = c - li * 4
                            if 0 <= j < 4:
                                nc.vector.tensor_tensor(
                                    out=pss, in0=pss, in1=masks[j],
                                    op=mybir.AluOpType.add,
                                )
                            ptile = pt_pool.tile([128, 512], F32R, tag="pt")
                            nc.scalar.activation(
                                ptile, pss, AF.Exp,
                                bias=(pb_sb if c >= 16 else 0.0), scale=1.0,
                            )
                            nc.tensor.matmul(
                                pso, vaug[:, c, :], ptile,
                                start=(idx == 0), stop=(idx == len(chunks) - 1),
                            )
                        # tail: transpose + normalize
                        osb = ob_pool.tile([H + 1, 512], F32, tag="osb")
                        nc.vector.tensor_copy(osb, pso)
                        rstage = ro_pool.tile([128, 4, H], F32, tag="ro")
                        for j in range(4):
                            pst2 = ps_t2.tile([128, H + 1], F32, tag="pst2")
                            nc.tensor.transpose(
                                pst2, osb[:, j * 128:(j + 1) * 128],
                                ident[0:H + 1, 0:H + 1],
                            )
                            rn = rn_pool.tile([128, H + 1], F32, tag="rn")
                            nc.vector.tensor_copy(rn, pst2)
                            rv = rv_pool.tile([128, 1], F32, tag="rv")
                            nc.vector.reciprocal(rv, rn[:, H:H + 1])
                            nc.vector.tensor_scalar_mul(
                                rstage[:, j, :], rn[:, 0:H], rv)
                        nc.sync.dma_start(
                            out=r_out[li * 512:(li + 1) * 512, :].rearrange(
                                "(j p) h -> p j h", p=128),
                            in_=rstage,
                        )

            if reps is None:
                body()
            else:
                with tc.For_i(0, reps, 1):
                    body()

    nc.compile()
    return nc


def _prep_inputs(x, wq_w, wq_b, wk_w, wk_b, wv_w, wv_b):
    x = np.asarray(x, np.float32)
    wkv = np.ascontiguousarray(
        np.concatenate([np.asarray(wk_w), np.asarray(wv_w)], axis=1), np.float32)
    wq = np.ascontiguousarray(np.asarray(wq_w), np.float32)
    bkv = np.ascontiguousarray(
        np.concatenate([np.asarray(wk_b), np.asarray(wv_b)]), np.float32
    ).reshape(2 * H, 1)
    bq8 = np.ascontiguousarray(
        np.asarray(wq_b) / 8.0, np.float32).reshape(H, 1)
    in_maps = []
    for c in range(8):
        b, h = c // 2, c % 2
        own = x[b, h * L:(h + 1) * L, :]
        other = x[b, (1 - h) * L:(2 - h) * L, :]
        xt = np.ascontiguousarray(np.concatenate([own, other], axis=0).T)
        pb = np.full((128, 1), 0.0 if h == 1 else NEG, np.float32)
        in_maps.append({
            "xt": xt, "wkv": wkv, "wq": wq, "bkv": bkv, "bq8": bq8,
            "pbias": pb,
        })
    return in_maps


def kernel(x, wq_w, wq_b, wk_w, wk_b, wv_w, wv_b):
    nc = build_nc()
    in_maps = _prep_inputs(x, wq_w, wq_b, wk_w, wk_b, wv_w, wv_b)
    res = bass_utils.run_bass_kernel_spmd(nc, in_maps, core_ids=list(range(8)))
    result = np.empty((B, S, H), np.float32)
    K = np.empty((B, S, H), np.float32)
    V = np.empty((B, S, H), np.float32)
    for c in range(8):
        b, h = c // 2, c % 2
        rows = slice(h * L, (h + 1) * L)
        result[b, rows] = res.results[c]["r_out"]
        K[b, rows] = res.results[c]["k_out"]
        V[b, rows] = res.results[c]["v_out"]
    return result, K, V
